# revision 111
# baseline (speedup 1.0000x reference)
"""Trainium2 Bass kernel for nn_Attention_43190191129190.

Model (per batch element b of 8):
    y   = x + dwconv3x3(x) + conv_b          (depthwise residual positional conv)
    qkv = y @ qkv_w.T ; split into q, k, v   (8 heads, dim 32)
    out = softmax(q k^T / sqrt(32)) v
    out = out @ out_w.T + out_b
Sharding: pure data-parallel, one batch element per NeuronCore (8 cores).

Per-core design (v6 — wide merged instructions, clean S->exp pipeline):

  Three engines are near-evenly loaded: PE (~51us: S dominates at the
  cost model's ap_size(out)*0.42ns), DVE+ACT (~94us combined: every S
  element must be exp'd and every PSUM result evacuated by ACT/DVE —
  DMA and GPSIMD cannot touch PSUM, and TRN2 matmuls cannot write
  16-bit PSUM, so no 2x-mode reads of S). fp8 DoubleRow was measured
  (host sim) to blow the 2e-2 gate (7e-2) — everything stays bf16/f32.

  1. x arrives [C, N] bf16 (host pre-transposed): ONE straight DMA per
     channel tile (each DMA costs ~2.5us of serial pipeline: HWDGE 625
     + DGE 650 + transfer + 900 completion sem — transposing on the
     host removes two xbar-transpose DMAs from the critical path).
     Interior copied into a zero-haloed [C, 34, 34] image.
  2. diag conv matrices: diag[c,t,f] = w18[c,t] * id[c,f] via one DVE
     tensor_tensor per channel tile (faster than Pool affine_selects).
  3. conv per (ct, j): 9 diagonal-matmul taps in one bank; conv_b is
     folded into the PSUM->SBUF evacuation as a per-partition bias
     (ACT Identity-activation / DVE broadcast add), not a K=1 matmul.
  4. q^T/k^T per feature tile: [128,2,512] tile, 4 matmuls, ONE
     1024-wide evacuation. v: two 4-token-chunk units with ONE strided
     evacuation each into [v_h|1] 33-wide head slots (ones preset).
     ALL of this runs before the attention loop: any PSUM allocation
     inside the m-loop steals one of the 3 rotating S slots and
     serializes the S->exp chain.
  5. Attention, head pair per generation, 8 m-steps each:
       S^T per (head, m): one [128,2,512] f32 PSUM tile (two 512-wide
       matmuls), then ONE 1024-wide exp. hs0 sits on the 1-step-slack
       PSUM slot and its exp gates the S-issue chain -> always ACT
       exact Exp (1038ns); hs1 has 2-step slack -> DVE Schraudolph
       (tensor_scalar s*A+B -> int16 bits == bf16(exp(s*SCALE))).
       PV: per-head [128, 8x33] PSUM accumulator, stationary p^T
       chunks, moving [v_h|1]; column 32 accumulates the softmax
       denominators; one accumulation group per bank.
       Normalization (deferred into the next pair's steps): the PV
       accumulator is staged PSUM->SBUF on ACT (freeing the bank),
       reciprocal on DVE from SBUF, and the broadcast multiply runs on
       the otherwise-idle GPSIMD/Pool engine -> a_sb bf16. The last
       pair normalizes directly on DVE (shortest tail latency).
  6. a_sb -> attnT: 8 transposes per ct share ONE [128,1024] bf16 bank
     (single accumulation group, disjoint regions), ONE 2x-mode DVE
     copy (chunk 1 mid-loop as the only in-loop extra; chunk 0 in the
     tail with the two copy halves split across DVE and ACT).
  7. projection in the tail only (no mid-kernel staging: a staged half
     would cost the same tail matmul time as computing it directly):
     per token-chunk pair one bank, K=128 matmuls for both feature
     chunks + one K=1 out_b tap, copies alternating ACT/DVE, one DMA
     per 2 chunks on alternating queues.

  PSUM: 3x[128,2,512] f32 rotating S slots + 2x[128,264] PV
  accumulators.
"""

import os

import numpy as np

import concourse.bass as bass
import concourse.tile as tile
from concourse import bacc, mybir
from concourse.bass_utils import run_bass_kernel_spmd

F32 = mybir.dt.float32
F32R = mybir.dt.float32r
BF16 = mybir.dt.bfloat16
I16 = mybir.dt.int16
AF = mybir.ActivationFunctionType
ALU = mybir.AluOpType

B, N, C = 8, 1024, 256
HEADS, DH = 8, 32
SCALE = DH ** -0.5
PAD = 34  # 32x32 spatial grid with 1-px halo

# blobA (bf16): id [128, 0:128] | w18 [128, 128:146] | convb cols [128, 146:148]
BA_ID, BA_W18, BA_CONVB, BAW = 0, 128, 146, 148
# blobB (bf16): outwT [128, 0:512] | outb row0 [512:768]
BB_OWT, BB_OUTB, BBW = 0, 512, 768

TAPS = [(ky, kx) for ky in range(3) for kx in range(3)]
# chunk-1 head pairs first so the chunk-1 projection can run mid-kernel;
# the tail then only waits on the last pair's (chunk-0) normalization
PAIRS = [(5, 7), (4, 6), (1, 3), (0, 2)]

# Schraudolph fast-exp: int16 bits of bf16(exp(s*SCALE)) = s*A + B
SCHR_C = 450000.0
SCHR_A = float(SCALE * (2 ** 23) / np.log(2) / 65536.0)
SCHR_B = float((127 * 2 ** 23 - SCHR_C) / 65536.0)


def build_nc(debug_dump=False):
    nc = bacc.Bacc("TRN2", target_bir_lowering=False, debug=False, num_devices=8)

    # x arrives host-pre-transposed: [C, N] bf16, one straight DMA
    x_d = nc.dram_tensor("x", (C, N), BF16, kind="ExternalInput").ap()
    qkvwT_d = nc.dram_tensor("qkv_wT", (C, 3 * C), F32R, kind="ExternalInput").ap()
    blobA_d = nc.dram_tensor("blobA", (128, BAW), BF16, kind="ExternalInput").ap()
    blobB_d = nc.dram_tensor("blobB", (128, BBW), BF16, kind="ExternalInput").ap()
    # output staged bf16 (halves the tail DMA drain); host upcasts to f32
    out_d = nc.dram_tensor("out", (N, C), BF16, kind="ExternalOutput").ap()
    dbg = {}
    if debug_dump:
        for name, shape in (
            ("d_yT", (128, 2, N)), ("d_qT", (128, 2, N)), ("d_kT", (128, 2, N)),
            ("d_v", (128, 8, 264)), ("d_asb", (128, 8, 256)),
        ):
            dbg[name] = nc.dram_tensor(name, shape, F32, kind="ExternalOutput").ap()

    with tile.TileContext(nc) as tc:
        with (
            tc.tile_pool(name="const", bufs=1) as const,
            tc.tile_pool(name="big", bufs=1) as big,
            tc.tile_pool(name="pT", bufs=16) as ppool,
            tc.tile_pool(name="rcp", bufs=6) as rcp_p,
            tc.tile_pool(name="outs", bufs=4) as outs_p,
            tc.tile_pool(name="pst", bufs=3, space="PSUM") as pst,
            tc.tile_pool(name="pap", bufs=2, space="PSUM") as pap,
        ):
            # ---- persistent activations (x image first: DMA critical path)
            xpadT = big.tile([128, 2, PAD * PAD], BF16, tag="xpadT")
            xpv = xpadT.bitcast(mybir.dt.uint16).rearrange(
                "p ct (h w) -> p ct h w", h=PAD
            )
            nc.vector.memset(xpv[:, :, 0, :], 0)
            nc.vector.memset(xpv[:, :, PAD - 1, :], 0)
            nc.vector.memset(xpv[:, :, :, 0], 0)
            nc.vector.memset(xpv[:, :, :, PAD - 1], 0)

            # ---- DMAs. Per-DMA cost in the serial DMA pipeline is large
            # (HWDGE 625 + DGE delay 650 + transfer + completion sem 900),
            # so x is host-pre-transposed and lands in ONE straight DMA.
            blobA_sb = const.tile([128, BAW], BF16, tag="blobA")
            nc.sync.dma_start(blobA_sb, blobA_d)
            xstg = big.tile([128, 2, N], BF16, tag="xstg")
            for ct in range(2):
                nc.sync.dma_start(xstg[:, ct, :],
                                  x_d[ct * 128:(ct + 1) * 128, :])
            id_sb = blobA_sb[:, BA_ID:BA_ID + 128]
            w18_sb = blobA_sb[:, BA_W18:BA_W18 + 18]
            convb2_sb = blobA_sb[:, BA_CONVB:BA_CONVB + 2]
            qkvwT_sb = const.tile([128, 2, 3 * C], F32R, tag="qkvwT")
            for ct in range(2):
                nc.sync.dma_start(
                    qkvwT_sb[:, ct, 0:512],
                    qkvwT_d[ct * 128:(ct + 1) * 128, 0:512],
                )
            nc.sync.dma_start(
                qkvwT_sb[:, :, 512:768],
                qkvwT_d[:, 512:768].rearrange("(kc p) f -> p kc f", p=128),
            )
            blobB_sb = const.tile([128, BBW], BF16, tag="blobB")
            nc.sync.dma_start(blobB_sb, blobB_d)
            outwT_sb = blobB_sb[:, BB_OWT:BB_OWT + 512].rearrange(
                "p (kc f) -> p kc f", kc=2)
            outb_sb = blobB_sb[0:1, BB_OUTB:BB_OUTB + 256]

            # diag conv matrices: diag[c, t, f] = w18[c, t] * id[c, f] via
            # one DVE tensor_tensor per channel tile (DVE is idle at startup
            # and this beats the Pool affine_select by ~3us of latency)
            # ---- warm-ups (after the DMA issues so they don't block the
            # ACT queue): the exp ACT-table load and a chained trickle of
            # tiny PE matmuls (keeps the PE "recently active" through the
            # DMA wait so the conv burst is not dispatched into the cost
            # model's cold p-state)
            zerob_sb = const.tile([128, 1], F32, tag="zerob")
            nc.vector.memset(zerob_sb, 0.0)
            warm_sb = const.tile([1, 1], F32, tag="warm")
            nc.scalar.activation(
                warm_sb, zerob_sb[0:1, 0:1], AF.Exp,
                bias=zerob_sb[0:1], scale=1.0,
            )
            wv = const.tile([1, 20], F32, tag="wv")
            nc.vector.memset(wv, 0.0)
            for k in range(17):
                wps = pst.tile([128, 2, 512], F32, tag="ps", name="wps")
                nc.tensor.matmul(
                    wps[0:1, 0, 0:1], lhsT=wv[0:1, k:k + 1],
                    rhs=wv[0:1, k:k + 1], start=True, stop=True,
                )
                if k + 1 < 20:
                    nc.scalar.copy(wv[0:1, k + 1:k + 2], wps[0:1, 0, 0:1])

            diag_sb = const.tile([128, 18, 128], BF16, tag="diag")

            def emit_diag(ct):
                idb = bass.AP(
                    tensor=id_sb.tensor, offset=id_sb.offset,
                    ap=[list(id_sb.ap[0]), [0, 9], [1, 128]],
                )
                w18b = bass.AP(
                    tensor=w18_sb.tensor,
                    offset=w18_sb.offset + ct * 9,
                    ap=[list(w18_sb.ap[0]), [1, 9], [0, 128]],
                )
                nc.vector.tensor_tensor(
                    out=diag_sb[:, ct * 9:(ct + 1) * 9, :],
                    in0=idb, in1=w18b, op=ALU.mult,
                )

            def emit_xpad(ct):
                nc.vector.tensor_copy(
                    xpadT[:, ct, :].rearrange("p (h w) -> p h w", h=PAD)[
                        :, 1:33, 1:33
                    ],
                    xstg[:, ct, :].rearrange("p (h w) -> p h w", h=32),
                )

            # ones row generated on device (proj-bias rhs)
            ones_sb = const.tile([1, 512], BF16, tag="ones")
            nc.gpsimd.memset(ones_sb, 1.0)
            # conv bias in f32 for the per-partition bias of the conv
            # evacuation (folds the bias add into the PSUM->SBUF copy)
            convbf = const.tile([128, 2], F32, tag="convbf")
            # DVE order matters: ct0's conv inputs complete before ct1's
            # begin, so the ct0 conv matmuls start ~2us sooner
            nc.vector.tensor_copy(convbf, convb2_sb)
            emit_diag(0)
            emit_xpad(0)
            emit_diag(1)
            emit_xpad(1)

            yT = big.tile([128, 2, N], F32R, tag="yT")
            qT = big.tile([128, 2, N], F32R, tag="qT")
            kT = big.tile([128, 2, N], F32R, tag="kT")
            # [v_h | 1] per (token-chunk, head); ones preset via memset
            vsb = big.tile([128, 8, 8 * 33], BF16, tag="v")
            nc.gpsimd.memset(vsb, 1.0)
            a_sb = big.tile([128, 8, 256], BF16, tag="a_sb")
            attnT = big.tile([128, 2, N], BF16, tag="attnT")

            # psum evacuations: GPSIMD cannot access PSUM on HW, so they
            # alternate between the ACT (scalar.copy) and DVE engines
            _cp = [0]

            def copy_alt(dst, src_ap):
                _cp[0] += 1
                if _cp[0] % 2:
                    nc.scalar.copy(dst, src_ap)
                else:
                    nc.vector.tensor_copy(dst, src_ap)

            # ---- conv: per (ct, j) half: 9 diagonal matmuls + K=1 bias tap,
            # one 512-wide evacuation (j-split so the attention wavefront can
            # start on the j0 token half while j1 is still convolving)
            def emit_conv_half(ct, j):
                cps = pst.tile([128, 512], F32, tag="ps", name=f"cacc{ct}{j}")
                view = xpadT[:, ct, :].rearrange("p (h w) -> p h w", h=PAD)
                for t, (ky, kx) in enumerate(TAPS):
                    nc.tensor.matmul(
                        cps,
                        lhsT=diag_sb[:, ct * 9 + t, :],
                        rhs=view[:, ky + 16 * j: ky + 16 * j + 16, kx: kx + 32],
                        start=(t == 0),
                        stop=(t == 8),
                    )
                # conv bias folded into the evacuation (per-partition add)
                dst = yT[:, ct, j * 512:(j + 1) * 512]
                _cp[0] += 1
                if _cp[0] % 2:
                    nc.scalar.activation(
                        dst, cps, AF.Identity,
                        bias=convbf[:, ct:ct + 1], scale=1.0)
                else:
                    cb = bass.AP(
                        tensor=convbf.tensor, offset=convbf.offset + ct,
                        ap=[list(convbf.ap[0]), [0, 512]],
                    )
                    nc.vector.tensor_tensor(
                        out=dst, in0=cps, in1=cb, op=ALU.add)

            # ---- q^T / k^T per feature tile: 4 matmuls, one evacuation ----
            def emit_qk(ft):
                dstT, dc = (qT, ft) if ft < 2 else (kT, ft - 2)
                fofs = 0 if ft < 2 else 256
                qps = pst.tile([128, 2, 512], F32, tag="ps", name="qps")
                for j in range(2):
                    for kc in range(2):
                        nc.tensor.matmul(
                            qps[:, j, :],
                            lhsT=qkvwT_sb[:, kc, fofs + dc * 128: fofs + (dc + 1) * 128],
                            rhs=yT[:, kc, j * 512:(j + 1) * 512],
                            start=(kc == 0),
                            stop=(kc == 1),
                        )
                copy_alt(dstT[:, dc, :], qps.rearrange("p a b -> p (a b)"))

            # ---- v: 4 token chunks per unit, 8 matmuls, one strided evac ----
            def emit_v4(u):
                vps = pst.tile([128, 2, 512], F32, tag="ps", name="vps")
                for q in range(4):
                    nt = u * 4 + q
                    dst = vps[:, q // 2, (q % 2) * 256:(q % 2) * 256 + 256]
                    for kc in range(2):
                        # one open accumulation group per bank: start on the
                        # bank's first write, stop on its last
                        nc.tensor.matmul(
                            dst,
                            lhsT=yT[:, kc, nt * 128:(nt + 1) * 128],
                            rhs=qkvwT_sb[:, kc, 512:768],
                            start=(kc == 0 and q % 2 == 0),
                            stop=(kc == 1 and q % 2 == 1),
                        )
                sv = vps.rearrange("p a (q hh c) -> p (a q) hh c", q=2, c=32)
                dv = vsb[:, u * 4:(u + 1) * 4, :].rearrange(
                    "p n (hh c) -> p n hh c", c=33)[:, :, :, 0:32]
                copy_alt(dv, sv)

            # pre-loop: exactly what pair 0 needs up front (chunk-1 q/k and
            # the first four v chunks); the rest trickles in as one light
            # half-unit extra per m-step so the S/exp PSUM rotation is never
            # starved for long
            for ct in range(2):
                for j in range(2):
                    emit_conv_half(ct, j)
            emit_qk(1)
            emit_qk(3)
            emit_v4(0)
            emit_v4(1)
            emit_qk(0)
            emit_qk(2)

            # ---- a_sb -> attnT: 8 transposes sharing one bank + ONE copy ----
            def emit_atr_mm(ct, nc_i, tp):
                nc.tensor.matmul(
                    tp[:, nc_i * 128:(nc_i + 1) * 128],
                    lhsT=a_sb[:, nc_i, ct * 128:(ct + 1) * 128],
                    rhs=id_sb,
                    is_transpose=True,
                    start=(nc_i == 0),
                    stop=(nc_i == 7),
                )

            # interleaved extras, one self-contained slice per m-step
            def emit_atr_ct(ct):
                # all 8 transposes share one bank-tile + ONE 2x-mode copy;
                # single slice keeps the PSUM slot hold under ~1 m-step
                tp = pst.tile([128, 1024], BF16, tag="ps", name=f"atp{ct}")
                for i in range(8):
                    emit_atr_mm(ct, i, tp)
                nc.vector.tensor_copy(attnT[:, ct, :], tp)

            def pair_extra(ip, m):
                if ip == 2:
                    if m == 6:
                        emit_atr_ct(1)

            # ---- merged exp: ONE 1024-wide instruction per (head, m).
            # hs0 sits on the 1-step-slack PSUM slot: its exp gates the
            # S-issue chain, so it always runs on the faster ACT engine.
            # hs1 (2-step slack) goes to DVE except two steps per pair,
            # balancing total engine busy (~42 ACT / 22 DVE tiles).
            def emit_exp_half(eng, sv, w):
                if eng == "A":
                    p = ppool.tile([128, w], BF16, tag="pT", name="pA")
                    nc.scalar.activation(p, sv, AF.Exp, bias=zerob_sb, scale=SCALE)
                    return p
                p = ppool.tile([128, w], I16, tag="pT", name="pV")
                nc.vector.tensor_scalar(
                    out=p, in0=sv, scalar1=SCHR_A, scalar2=SCHR_B,
                    op0=ALU.mult, op1=ALU.add,
                )
                return p.bitcast(BF16)

            def emit_exp(eng, st2):
                return emit_exp_half(
                    eng, st2.rearrange("p a b -> p (a b)"), 1024)

            # ---- attention ----
            def emit_pv(m, ph, pas, heads, rng=None):
                # one accumulation group per pa bank: start only on the first
                # write (lazy 2KB region-zeroing covers the other 7
                # sub-regions), stop only on the last. rng selects a 4-chunk
                # n-range for the pair-0 wavefront half-tiles.
                base = 0 if rng is None else rng
                for nc_i in (range(8) if rng is None else range(rng, rng + 4)):
                    for hs in (0, 1):
                        nc.tensor.matmul(
                            pas[hs][:, nc_i * 33: nc_i * 33 + 33],
                            lhsT=ph[hs][:, (nc_i - base) * 128:
                                        (nc_i - base + 1) * 128],
                            rhs=vsb[:, m, 33 * heads[hs]: 33 * heads[hs] + 33],
                            start=(m == 0 and nc_i == 0),
                            stop=(m == 7 and nc_i == 7),
                        )

            def emit_norm(pas, heads, via_pool=False):
                for h, pa in zip(heads, pas):
                    pav = pa.rearrange("p (nc e) -> p nc e", e=33)
                    rcp = rcp_p.tile([128, 8], F32, tag="rcp", name="rcp")
                    if via_pool:
                        # DVE is the loop's ceiling engine: stage the PSUM
                        # accumulator to SBUF on ACT, then run reciprocal +
                        # broadcast multiply on the otherwise-idle Pool
                        psb = rcp_p.tile([128, 264], F32, tag="nrm",
                                         name="psb")
                        nc.scalar.copy(psb, pa)
                        pv = psb.rearrange("p (nc e) -> p nc e", e=33)
                        nc.vector.reciprocal(rcp, pv[:, :, 32])
                        rcp_b = bass.AP(
                            tensor=rcp.tensor, offset=rcp.offset,
                            ap=[list(rcp.ap[0]), [1, 8], [0, 32]],
                        )
                        nc.gpsimd.tensor_tensor(
                            out=a_sb[:, :, h * 32: h * 32 + 32],
                            in0=pv[:, :, 0:32],
                            in1=rcp_b,
                            op=ALU.mult,
                        )
                    else:
                        nc.vector.reciprocal(rcp, pav[:, :, 32])
                        rcp_b = bass.AP(
                            tensor=rcp.tensor, offset=rcp.offset,
                            ap=[list(rcp.ap[0]), [1, 8], [0, 32]],
                        )
                        nc.vector.tensor_tensor(
                            out=a_sb[:, :, h * 32: h * 32 + 32],
                            in0=pav[:, :, 0:32],
                            in1=rcp_b,
                            op=ALU.mult,
                        )

            def emit_s_full(h, m):
                a = 32 * (h % 4)
                hc = h // 4
                st2 = pst.tile([128, 2, 512], F32, tag="ps", name="st")
                for j in range(2):
                    nc.tensor.matmul(
                        st2[:, j, :],
                        lhsT=kT[a:a + 32, hc, m * 128:(m + 1) * 128],
                        rhs=qT[a:a + 32, hc, j * 512:(j + 1) * 512],
                        start=True,
                        stop=True,
                        tile_position=(a, 0),
                    )
                return st2

            carry = []
            for ip, (hA, hB) in enumerate(PAIRS):
                pas = (
                    pap.tile([128, 264], F32, tag="pa", name=f"paA{ip}"),
                    pap.tile([128, 264], F32, tag="pa", name=f"paB{ip}"),
                )
                heads = (hA, hB)
                pend = []
                for m in range(8):
                    ph = {
                        hs: emit_exp(
                            "A" if hs == 0 else "V", emit_s_full(h, m))
                        for hs, h in ((0, hA), (1, hB))
                    }
                    pend.append((m, ph, None))
                    # carried PVs wait until m>=2 so the previous pair's
                    # trailing exps (still draining on DVE) don't head-of-line
                    # stall the PE queue
                    if carry and m >= 3:
                        carry.pop(0)()
                    pair_extra(ip, m)
                    # the last pair drains its PVs earlier to shorten the tail
                    if len(pend) > (1 if ip == 3 else 2):
                        e = pend.pop(0)
                        emit_pv(e[0], e[1], pas, heads, e[2])
                # defer the tail PVs + normalization into the next pair's
                # m-loop so the PE never waits on the trailing exps
                thunks = [
                    (lambda e=e, pas=pas, heads=heads: emit_pv(
                        e[0], e[1], pas, heads, e[2]))
                    for e in pend
                ]
                for hs in (0, 1):
                    thunks.append(
                        lambda hs=hs, pas=pas, heads=heads, ip=ip: emit_norm(
                            (pas[hs],), (heads[hs],), via_pool=(ip < 3))
                    )
                carry = thunks

            # ---- tail: last pair's PVs + norms first (they gate the whole
            # output chain) ----
            for t in carry:  # PV(7), the two norms
                t()

            if debug_dump:
                nc.sync.dma_start(dbg["d_yT"], yT.bitcast(F32))
                nc.sync.dma_start(dbg["d_qT"], qT.bitcast(F32))
                nc.sync.dma_start(dbg["d_kT"], kT.bitcast(F32))
                dvf = big.tile([128, 8, 264], F32, tag="dvf")
                nc.vector.tensor_copy(dvf, vsb)
                nc.sync.dma_start(dbg["d_v"], dvf)
                daf = big.tile([128, 8, 256], F32, tag="daf")
                nc.vector.tensor_copy(daf, a_sb)
                nc.sync.dma_start(dbg["d_asb"], daf)

            # transpose chunk-0 (shared-bank, half-copies on BOTH engines so
            # the first projections start before the second half lands),
            # project in token-chunk pairs, merged copies, store
            tp0 = pst.tile([128, 1024], BF16, tag="ps", name="atp0")
            for i in range(8):
                emit_atr_mm(0, i, tp0)
            nc.vector.tensor_copy(attnT[:, 0, 0:512], tp0[:, 0:512])
            nc.scalar.copy(attnT[:, 0, 512:1024], tp0[:, 512:1024])
            for np_ in range(4):
                ops = pst.tile([128, 2, 512], F32, tag="ps", name="ops")
                for q in range(2):
                    nt = np_ * 2 + q
                    dst = ops[:, 0, q * 256:(q + 1) * 256]
                    for kc in range(2):
                        nc.tensor.matmul(
                            dst,
                            lhsT=attnT[:, kc, nt * 128:(nt + 1) * 128],
                            rhs=outwT_sb[:, kc, :],
                            start=(q == 0 and kc == 0),
                            stop=False,
                        )
                # out_b as a K=1 tap over the whole pair bank
                ob = bass.AP(
                    tensor=outb_sb.tensor, offset=outb_sb.offset,
                    ap=[list(outb_sb.ap[0]), [0, 2], [1, 256]],
                )
                nc.tensor.matmul(
                    ops[:, 0, :],
                    lhsT=ones_sb[0:1, 0:128],
                    rhs=ob,
                    start=False,
                    stop=True,
                )
                osb2 = outs_p.tile([128, 2, C], BF16, tag="o", name="osb2")
                # alternate engines: DVE is idle once the last norms are done
                copy_alt(osb2.rearrange("p a b -> p (a b)"), ops[:, 0, :])
                # one batched DMA per 2 token chunks (HWDGE overhead is
                # per-descriptor-set, ~625ns each)
                oq = nc.sync if np_ % 2 else nc.scalar
                oq.dma_start(
                    out_d[np_ * 256:(np_ + 1) * 256, :].rearrange(
                        "(c p) f -> p c f", p=128),
                    osb2,
                )

    nc.compile()
    return nc


_NC = None
LAST_RESULTS = None


def _host_prep(conv_w, conv_b, qkv_w, out_w, out_b):
    import ml_dtypes

    conv_w = np.asarray(conv_w, np.float32).reshape(C, 3, 3)
    w18 = np.zeros((128, 18), np.float32)
    for ct in range(2):
        for t, (ky, kx) in enumerate(TAPS):
            d = conv_w[128 * ct: 128 * (ct + 1), ky, kx].copy()
            if (ky, kx) == (1, 1):
                d += 1.0  # residual connection folded into the center tap
            w18[:, ct * 9 + t] = d
    blobA = np.zeros((128, BAW), ml_dtypes.bfloat16)
    blobA[:, BA_ID:BA_ID + 128] = np.eye(128, dtype=ml_dtypes.bfloat16)
    blobA[:, BA_W18:BA_W18 + 18] = w18.astype(ml_dtypes.bfloat16)
    cb = np.asarray(conv_b, np.float32).reshape(2, 128).T
    blobA[:, BA_CONVB:BA_CONVB + 2] = cb.astype(ml_dtypes.bfloat16)
    blobB = np.zeros((128, BBW), ml_dtypes.bfloat16)
    owT = np.ascontiguousarray(np.asarray(out_w, np.float32).T).astype(
        ml_dtypes.bfloat16)  # [256 in, 256 outc]
    blobB[:, BB_OWT:BB_OWT + 512] = np.concatenate(
        [owT[0:128, :], owT[128:256, :]], axis=1)
    blobB[0, BB_OUTB:BB_OUTB + 256] = np.asarray(out_b, np.float32).astype(
        ml_dtypes.bfloat16)
    return {
        "qkv_wT": np.ascontiguousarray(np.asarray(qkv_w, np.float32).T),
        "blobA": blobA,
        "blobB": blobB,
    }


def _prep_x(x):
    """bf16, host-transposed to [B, C, N] for straight (transpose-free) DMA."""
    import ml_dtypes

    xt = np.swapaxes(np.asarray(x, np.float32), -1, -2)
    return np.ascontiguousarray(xt.astype(ml_dtypes.bfloat16))


def kernel(x, conv_w, conv_b, qkv_w, out_w, out_b):
    global _NC, LAST_RESULTS

    if _NC is None:
        _NC = build_nc()
    x = _prep_x(x)
    shared = _host_prep(conv_w, conv_b, qkv_w, out_w, out_b)
    in_maps = [{**shared, "x": np.ascontiguousarray(x[b])} for b in range(B)]
    trace = bool(int(os.environ.get("KERNEL_TRACE", "0")))
    try:
        res = run_bass_kernel_spmd(_NC, in_maps, core_ids=list(range(B)), trace=trace)
    except Exception:
        if not trace:
            raise
        res = run_bass_kernel_spmd(_NC, in_maps, core_ids=list(range(B)), trace=False)
    LAST_RESULTS = res
    return np.stack([res.results[b]["out"] for b in range(B)], axis=0).astype(
        np.float32)
